# revision 5
# baseline (speedup 1.0000x reference)
"""Trainium2 Bass kernel for nn_AttentionSampler.

reference:  energies = sites @ w_site + (local . w_local) + b ; softmax(energies)
Softmax is invariant to the additive constant, so only sites @ attn_w[D:2D]
matters.

v3: TensorEngine matvec + bf16, all bulk DMA on SWDGE, no collectives.

- Host pre-casts sites to bf16 (tolerance is 2e-2; bf16 input rounding gives
  ~2e-3) and pre-transposes into 128x128 blocks so each block is a ready-made
  stationary operand: X[p, b*256 + h*128 + m] = sites[b*128 + m, h*128 + p].
  This halves HBM traffic (32MB/core) and moves all dot products to the PE
  array.
- Device per core: stream bf16 chunks via SWDGE only (HWDGE rings measured
  ~60-85 GB/s vs SWDGE bursts at ~430 GB/s; one slow HWDGE chunk in the
  middle of the block order stalls the PE and backs up the whole tile pool).
  First/last chunks are small so the PE starts early and the tail is short.
- Per 128-site block b: two accumulating matmuls (K=128 halves of D=256) with
  w halves as the moving operand -> energies land in PSUM [128, b] f32.
  Chunks alternate between two full PSUM banks so the per-chunk ACT exp
  (PSUM->SBUF) never touches the bank the PE is currently writing
  (same-bank PE-write + ACT-read is serialized by Tile).
- Energies are small (|e| < ~3 for this input distribution; fp32 exp is safe
  to |e| < 88) so no max-subtraction pass is needed.
- Host unshard: S = sum of all exp values (f64), output = exp/S. The 8 ranks
  never synchronize on device - no collectives, no rank-skew coupling.
"""

import sys

if "/opt/trn_rl_repo" not in sys.path:
    sys.path.insert(0, "/opt/trn_rl_repo")

import numpy as np

D = 256
N = 500000
N_CORES = 8
P = 128                    # SBUF/PSUM partitions; also sites per block
B = 490                    # blocks per core
SITES_CORE = P * B         # 62720
N_PAD = N_CORES * SITES_CORE  # 501760 (1760 zero-pad rows, discarded on host)
# blocks per chunk: small head chunk (fast PE start), small tail chunks
CHUNKS = [16] + [34] * 13 + [16, 16]
assert sum(CHUNKS) == B
BUFS = 6

_nc_cache = None


def build_nc():
    from concourse import bacc, mybir, tile

    f32 = mybir.dt.float32
    bf16 = mybir.dt.bfloat16
    nc = bacc.Bacc(
        "TRN2",
        target_bir_lowering=False,
        debug=False,
        enable_asserts=False,
        num_devices=N_CORES,
        num_swdge_queues=4,
    )
    sitesT = nc.dram_tensor("sitesT", [P, B * 2 * P], bf16, kind="ExternalInput")
    # w halves padded to 1KB/partition so the DMA engages all 16 HWDGE slots.
    wt = nc.dram_tensor("wt", [P, 512], bf16, kind="ExternalInput")
    out = nc.dram_tensor("out", [P * B], f32, kind="ExternalOutput")
    out_r = out.ap().rearrange("(p b) -> p b", p=P)

    AF = mybir.ActivationFunctionType
    MAXCH = max(CHUNKS)

    with tile.TileContext(nc) as tc:
        with (
            tc.tile_pool(name="loads", bufs=BUFS) as loads,
            tc.tile_pool(name="consts", bufs=1) as consts,
            tc.tile_pool(name="psum", bufs=1, space="PSUM") as psum_pool,
        ):
            w_tile = consts.tile([P, 512], bf16)
            nc.sync.dma_start(w_tile[:], wt.ap()[:, :])

            # Warm the ACT exp table (~2.7us one-time) under chunk 0's DMA.
            warm = consts.tile([1, 8], f32)
            nc.vector.memset(warm[:], 0.0)
            nc.scalar.activation(warm[:], warm[:], AF.Exp, scale=1.0)

            # Two full PSUM banks; chunks alternate so ACT exp on one bank
            # overlaps PE accumulation into the other.
            psums = [
                psum_pool.tile([P, 512], f32, name="psumA"),
                psum_pool.tile([P, 512], f32, name="psumB"),
            ]
            bank_off = [0, 0]
            outv = consts.tile([P, B], f32)

            b0 = 0
            for ci, nb in enumerate(CHUNKS):
                cols = nb * 2 * P
                t = loads.tile([P, MAXCH * 2 * P], bf16, tag="chunk")
                src = sitesT.ap()[:, b0 * 2 * P:(b0 + nb) * 2 * P]
                inst = nc.gpsimd.dma_start(t[:, 0:cols], src)
                qn = ci % 4
                if qn:
                    inst.ins.queue = f"qPoolDynamic{qn}"

                bank = ci % 2
                pt = psums[bank]
                off = bank_off[bank]
                for rb in range(nb):
                    col0 = rb * 2 * P
                    # 4-way column tiling: four 32-col strips of the
                    # stationary load+multiply concurrently on disjoint
                    # 32x32 sub-array column groups (concurrent LDWEIGHTS
                    # streams over separate XBUSes).
                    for h in range(2):
                        for j in range(4):
                            nc.tensor.matmul(
                                pt[32 * j:32 * j + 32, off + rb:off + rb + 1],
                                t[:, col0 + h * P + 32 * j:
                                     col0 + h * P + 32 * j + 32],
                                w_tile[:, h:h + 1],
                                start=(h == 0), stop=(h == 1),
                                tile_position=(0, 32 * j),
                            )
                # exp for this chunk's finished columns (PSUM -> SBUF).
                nc.scalar.activation(
                    outv[:, b0:b0 + nb],
                    pt[:, off:off + nb],
                    AF.Exp, scale=1.0,
                )
                bank_off[bank] += nb
                b0 += nb
            # single contiguous 251KB store (per-chunk column-slice stores
            # fragment into 128 tiny descriptors each and crawl at ~3GB/s)
            nc.scalar.dma_start(out_r, outv[:])

    nc.compile()
    return nc


def _get_nc():
    global _nc_cache
    if _nc_cache is None:
        _nc_cache = build_nc()
    return _nc_cache


def make_in_maps(sites, attn_w):
    import ml_dtypes

    bf = ml_dtypes.bfloat16
    sites = np.asarray(sites, dtype=np.float32)
    w = np.asarray(attn_w, dtype=np.float32)[D:2 * D].astype(bf)

    wt = np.zeros((P, 512), dtype=bf)
    wt[:, 0] = w[0:P]
    wt[:, 1] = w[P:2 * P]

    sp = np.zeros((N_PAD, D), dtype=bf)
    sp[:N] = sites.astype(bf)

    maps = []
    for c in range(N_CORES):
        shard = sp[c * SITES_CORE:(c + 1) * SITES_CORE]
        # [b, m, h, p] -> [p, b, h, m]
        R = shard.reshape(B, P, 2, P)
        X = np.ascontiguousarray(R.transpose(3, 0, 2, 1)).reshape(P, B * 2 * P)
        maps.append({"sitesT": X, "wt": wt})
    return maps


def kernel(local, sites, attn_w, attn_b):
    from concourse.bass_utils import run_bass_kernel_spmd

    nc = _get_nc()
    in_maps = make_in_maps(sites, attn_w)
    res = run_bass_kernel_spmd(nc, in_maps, list(range(N_CORES)))
    # out[m, b] holds exp(energy) of local site b*128+m -> transpose to site
    # order, drop the padding, normalize by the global sum.
    exps = [
        np.asarray(res.results[c]["out"], dtype=np.float32)
        .reshape(P, B).T.reshape(-1)
        for c in range(N_CORES)
    ]
    full = np.concatenate(exps)[:N]
    S = full.sum(dtype=np.float64)
    return (full / S).astype(np.float32)


# revision 6
# speedup vs baseline: 1.6110x; 1.6110x over previous
"""Trainium2 Bass kernel for nn_AttentionSampler.

reference:  energies = sites @ w_site + (local . w_local) + b ; softmax(energies)
Softmax is invariant to the additive constant, so only sites @ attn_w[D:2D]
matters.

v3: TensorEngine matvec + bf16, all bulk DMA on SWDGE, no collectives.

- Host pre-casts sites to bf16 (tolerance is 2e-2; bf16 input rounding gives
  ~2e-3) and pre-transposes into 128x128 blocks so each block is a ready-made
  stationary operand: X[p, b*256 + h*128 + m] = sites[b*128 + m, h*128 + p].
  This halves HBM traffic (32MB/core) and moves all dot products to the PE
  array.
- Device per core: stream bf16 chunks via SWDGE only (HWDGE rings measured
  ~60-85 GB/s vs SWDGE bursts at ~430 GB/s; one slow HWDGE chunk in the
  middle of the block order stalls the PE and backs up the whole tile pool).
  First/last chunks are small so the PE starts early and the tail is short.
- Per 128-site block b: two accumulating matmuls (K=128 halves of D=256) with
  w halves as the moving operand -> energies land in PSUM [128, b] f32.
  Chunks alternate between two full PSUM banks so the per-chunk ACT exp
  (PSUM->SBUF) never touches the bank the PE is currently writing
  (same-bank PE-write + ACT-read is serialized by Tile).
- Energies are small (|e| < ~3 for this input distribution; fp32 exp is safe
  to |e| < 88) so no max-subtraction pass is needed.
- Host unshard: S = sum of all exp values (f64), output = exp/S. The 8 ranks
  never synchronize on device - no collectives, no rank-skew coupling.
"""

import sys

if "/opt/trn_rl_repo" not in sys.path:
    sys.path.insert(0, "/opt/trn_rl_repo")

import numpy as np

D = 256
N = 500000
N_CORES = 8
P = 128                    # SBUF/PSUM partitions; also sites per block
B = 490                    # blocks per core
SITES_CORE = P * B         # 62720
N_PAD = N_CORES * SITES_CORE  # 501760 (1760 zero-pad rows, discarded on host)
# blocks per chunk: small head chunk (fast PE start), small tail chunks
CHUNKS = [16] + [34] * 13 + [16, 16]
assert sum(CHUNKS) == B
BUFS = 6

_nc_cache = None


def build_nc():
    from concourse import bacc, mybir, tile

    f32 = mybir.dt.float32
    bf16 = mybir.dt.bfloat16
    nc = bacc.Bacc(
        "TRN2",
        target_bir_lowering=False,
        debug=False,
        enable_asserts=False,
        num_devices=N_CORES,
        num_swdge_queues=4,
    )
    sitesT = nc.dram_tensor("sitesT", [P, B * 2 * P], bf16, kind="ExternalInput")
    # w halves padded to 1KB/partition so the DMA engages all 16 HWDGE slots.
    wt = nc.dram_tensor("wt", [P, 512], bf16, kind="ExternalInput")
    out = nc.dram_tensor("out", [P * B], f32, kind="ExternalOutput")
    out_r = out.ap().rearrange("(p b) -> p b", p=P)

    AF = mybir.ActivationFunctionType
    MAXCH = max(CHUNKS)

    with tile.TileContext(nc) as tc:
        with (
            tc.tile_pool(name="loads", bufs=BUFS) as loads,
            tc.tile_pool(name="consts", bufs=1) as consts,
            tc.tile_pool(name="psum", bufs=1, space="PSUM") as psum_pool,
        ):
            w_tile = consts.tile([P, 512], bf16)
            nc.sync.dma_start(w_tile[:], wt.ap()[:, :])

            # Warm the ACT exp table (~2.7us one-time) under chunk 0's DMA.
            warm = consts.tile([1, 8], f32)
            nc.vector.memset(warm[:], 0.0)
            nc.scalar.activation(warm[:], warm[:], AF.Exp, scale=1.0)

            # Two full PSUM banks; chunks alternate so ACT exp on one bank
            # overlaps PE accumulation into the other.
            psums = [
                psum_pool.tile([P, 512], f32, name="psumA"),
                psum_pool.tile([P, 512], f32, name="psumB"),
            ]
            bank_off = [0, 0]
            outv = consts.tile([P, B], f32)

            b0 = 0
            for ci, nb in enumerate(CHUNKS):
                cols = nb * 2 * P
                t = loads.tile([P, MAXCH * 2 * P], bf16, tag="chunk")
                src = sitesT.ap()[:, b0 * 2 * P:(b0 + nb) * 2 * P]
                inst = nc.gpsimd.dma_start(t[:, 0:cols], src)
                qn = ci % 4
                if qn:
                    inst.ins.queue = f"qPoolDynamic{qn}"

                bank = ci % 2
                pt = psums[bank]
                off = bank_off[bank]
                for rb in range(nb):
                    col0 = rb * 2 * P
                    # NOTE: LDWEIGHTS has a ~100ns fixed floor regardless of
                    # column count (32-col strips cost the same as 128), so
                    # column tiling only multiplies LDW count - keep one full
                    # 128-col stationary per half.
                    nc.tensor.matmul(
                        pt[:, off + rb:off + rb + 1],
                        t[:, col0:col0 + P],
                        w_tile[:, 0:1],
                        start=True, stop=False,
                    )
                    nc.tensor.matmul(
                        pt[:, off + rb:off + rb + 1],
                        t[:, col0 + P:col0 + 2 * P],
                        w_tile[:, 1:2],
                        start=False, stop=True,
                    )
                # exp for this chunk's finished columns (PSUM -> SBUF).
                nc.scalar.activation(
                    outv[:, b0:b0 + nb],
                    pt[:, off:off + nb],
                    AF.Exp, scale=1.0,
                )
                bank_off[bank] += nb
                b0 += nb
            # single contiguous 251KB store (per-chunk column-slice stores
            # fragment into 128 tiny descriptors each and crawl at ~3GB/s)
            nc.scalar.dma_start(out_r, outv[:])

    nc.compile()
    return nc


def _get_nc():
    global _nc_cache
    if _nc_cache is None:
        _nc_cache = build_nc()
    return _nc_cache


def make_in_maps(sites, attn_w):
    import ml_dtypes

    bf = ml_dtypes.bfloat16
    sites = np.asarray(sites, dtype=np.float32)
    w = np.asarray(attn_w, dtype=np.float32)[D:2 * D].astype(bf)

    wt = np.zeros((P, 512), dtype=bf)
    wt[:, 0] = w[0:P]
    wt[:, 1] = w[P:2 * P]

    sp = np.zeros((N_PAD, D), dtype=bf)
    sp[:N] = sites.astype(bf)

    maps = []
    for c in range(N_CORES):
        shard = sp[c * SITES_CORE:(c + 1) * SITES_CORE]
        # [b, m, h, p] -> [p, b, h, m]
        R = shard.reshape(B, P, 2, P)
        X = np.ascontiguousarray(R.transpose(3, 0, 2, 1)).reshape(P, B * 2 * P)
        maps.append({"sitesT": X, "wt": wt})
    return maps


def kernel(local, sites, attn_w, attn_b):
    from concourse.bass_utils import run_bass_kernel_spmd

    nc = _get_nc()
    in_maps = make_in_maps(sites, attn_w)
    res = run_bass_kernel_spmd(nc, in_maps, list(range(N_CORES)))
    # out[m, b] holds exp(energy) of local site b*128+m -> transpose to site
    # order, drop the padding, normalize by the global sum.
    exps = [
        np.asarray(res.results[c]["out"], dtype=np.float32)
        .reshape(P, B).T.reshape(-1)
        for c in range(N_CORES)
    ]
    full = np.concatenate(exps)[:N]
    S = full.sum(dtype=np.float64)
    return (full / S).astype(np.float32)


# revision 7
# speedup vs baseline: 1.6568x; 1.0285x over previous
"""Trainium2 Bass kernel for nn_AttentionSampler.

reference:  energies = sites @ w_site + (local . w_local) + b ; softmax(energies)
Softmax is invariant to the additive constant, so only sites @ attn_w[D:2D]
matters.

v6: bf16, SWDGE-only bulk DMA (~420 GB/s/core sustained), PE matvec + DVE
assist, no collectives.

- Host pre-casts sites to bf16 (tolerance 2e-2; bf16 gives ~2e-3) halving HBM
  traffic to 32MB/core, and splits each core's shard in two regions:
    * PE region (336 blocks x 128 sites): pre-transposed 128x128 blocks,
      X[p, b*256 + h*128 + m] = sites[b*128+m, h*128+p]. Each block is a
      stationary operand; two accumulating matmuls per block (K=128 halves of
      D=256, w halves as the 1-column moving operand) put energies in PSUM.
      LDWEIGHTS has a ~100ns floor per 128x128 stationary -> PE sustains
      ~1.33ns/site (measured 85ns per LDW+MM pair).
    * DVE region (19712 sites, natural [p, g, d] layout): per 256-col group a
      scalar_tensor_tensor (site row x broadcast w, bf16 2x mode) with
      accum_out produces 128 energies -> ~2.9ns/site on the otherwise-idle
      vector engine.
  Together the engines consume ~550 GB/s > the ~420 GB/s DMA stream, so
  compute tracks the stream instead of lagging it (PE alone is only ~270GB/s).
- PSUM: chunks alternate between two full banks so the per-chunk ACT exp
  never reads the bank the PE is writing (same-bank access serializes).
- Energies are small (|e| < ~3 here; fp32 exp is safe to 88): no max pass.
- One contiguous 251KB store of exp values; host computes the global sum and
  1/S scale during unshard. No collectives -> no rank-skew coupling.
"""

import sys

if "/opt/trn_rl_repo" not in sys.path:
    sys.path.insert(0, "/opt/trn_rl_repo")

import numpy as np

D = 256
N = 500000
N_CORES = 8
P = 128                     # SBUF/PSUM partitions
B_PE = 336                  # 128-site blocks in the PE region
S_PE = P * B_PE             # 43008 sites
G_DVE = 154                 # 256-wide groups in the DVE region
S_DVE = P * G_DVE           # 19712 sites
SITES_CORE = S_PE + S_DVE   # 62720
B = B_PE + G_DVE            # 490 output columns [128, 490]
N_PAD = N_CORES * SITES_CORE  # 501760 (1760 zero-pad rows, dropped on host)

CHUNKS_PE = [8, 8] + [34] * 8 + [28, 20]     # 336 blocks, 12 chunks
CHUNKS_DVE = [22] * 6 + [12, 10]             # 154 groups, 8 chunks
assert sum(CHUNKS_PE) == B_PE and sum(CHUNKS_DVE) == G_DVE
# (kind, index-into-kind-list); keeps PE:DVE bytes ~2.2:1 along the stream,
# small chunks first (fast engine start) and last (short tail)
SCHEDULE = [
    ("pe", 0), ("pe", 1), ("dve", 0), ("pe", 2), ("pe", 3), ("dve", 1),
    ("pe", 4), ("pe", 5), ("dve", 2), ("pe", 6), ("pe", 7), ("dve", 3),
    ("pe", 8), ("pe", 9), ("dve", 4), ("dve", 5), ("dve", 6), ("pe", 10),
    ("dve", 7), ("pe", 11),
]
assert sorted(i for k, i in SCHEDULE if k == "pe") == list(range(len(CHUNKS_PE)))
assert sorted(i for k, i in SCHEDULE if k == "dve") == list(range(len(CHUNKS_DVE)))

_nc_cache = None


def build_nc():
    from concourse import bacc, mybir, tile

    f32 = mybir.dt.float32
    bf16 = mybir.dt.bfloat16
    ALU = mybir.AluOpType
    AF = mybir.ActivationFunctionType
    nc = bacc.Bacc(
        "TRN2",
        target_bir_lowering=False,
        debug=False,
        enable_asserts=False,
        num_devices=N_CORES,
        num_swdge_queues=4,
    )
    sitesT = nc.dram_tensor("sitesT", [P, B_PE * 2 * P], bf16, kind="ExternalInput")
    sitesN = nc.dram_tensor("sitesN", [P, G_DVE * D], bf16, kind="ExternalInput")
    # w inputs padded to >=0.5KB/partition so DMAs engage all 16 engine slots
    wt = nc.dram_tensor("wt", [P, 512], bf16, kind="ExternalInput")
    out = nc.dram_tensor("out", [P * B], f32, kind="ExternalOutput")
    out_r = out.ap().rearrange("(p b) -> p b", p=P)

    MAXPE = max(CHUNKS_PE)
    MAXDVE = max(CHUNKS_DVE)

    with tile.TileContext(nc) as tc:
        with (
            tc.tile_pool(name="loads_pe", bufs=4) as loads_pe,
            tc.tile_pool(name="loads_dve", bufs=3) as loads_dve,
            tc.tile_pool(name="scratch", bufs=2) as scratch,
            tc.tile_pool(name="consts", bufs=1) as consts,
            tc.tile_pool(name="psum", bufs=1, space="PSUM") as psum_pool,
        ):
            w_tile = consts.tile([P, 512], bf16)
            nc.sync.dma_start(w_tile[:], wt.ap()[:, :])
            # broadcast w for the DVE path: wrep[p, d] = w[d]
            # (built from wt cols 2..257 which the host fills with w tiled)
            w_rep = w_tile[:, 2:2 + D]

            # Warm the ACT exp table (~2.7us one-time) under chunk 0's DMA.
            warm = consts.tile([1, 8], f32)
            nc.vector.memset(warm[:], 0.0)
            nc.scalar.activation(warm[:], warm[:], AF.Exp, scale=1.0)

            psums = [
                psum_pool.tile([P, 512], f32, name="psumA"),
                psum_pool.tile([P, 512], f32, name="psumB"),
            ]
            bank_off = [0, 0]
            outv = consts.tile([P, B], f32)
            energN = consts.tile([P, G_DVE], f32)

            pe_b0 = 0     # PE block cursor
            dve_g0 = 0    # DVE group cursor
            for ci, (kind, idx) in enumerate(SCHEDULE):
                qn = ci % 4
                if kind == "pe":
                    nb = CHUNKS_PE[idx]
                    t = loads_pe.tile([P, MAXPE * 2 * P], bf16, tag="pechunk")
                    src = sitesT.ap()[:, pe_b0 * 2 * P:(pe_b0 + nb) * 2 * P]
                    inst = nc.gpsimd.dma_start(t[:, 0:nb * 2 * P], src)
                    if qn:
                        inst.ins.queue = f"qPoolDynamic{qn}"
                    bank = ci % 2
                    pt = psums[bank]
                    off = bank_off[bank]
                    for rb in range(nb):
                        col0 = rb * 2 * P
                        nc.tensor.matmul(
                            pt[:, off + rb:off + rb + 1],
                            t[:, col0:col0 + P],
                            w_tile[:, 0:1],
                            start=True, stop=False,
                        )
                        nc.tensor.matmul(
                            pt[:, off + rb:off + rb + 1],
                            t[:, col0 + P:col0 + 2 * P],
                            w_tile[:, 1:2],
                            start=False, stop=True,
                        )
                    # exp of this chunk's finished PSUM columns -> SBUF
                    nc.scalar.activation(
                        outv[:, pe_b0:pe_b0 + nb],
                        pt[:, off:off + nb],
                        AF.Exp, scale=1.0,
                    )
                    bank_off[bank] += nb
                    pe_b0 += nb
                else:
                    ng = CHUNKS_DVE[idx]
                    t = loads_dve.tile([P, MAXDVE * D], bf16, tag="dvechunk")
                    src = sitesN.ap()[:, dve_g0 * D:(dve_g0 + ng) * D]
                    inst = nc.gpsimd.dma_start(t[:, 0:ng * D], src)
                    if qn:
                        inst.ins.queue = f"qPoolDynamic{qn}"
                    for g in range(ng):
                        prod = scratch.tile([P, D], bf16, tag="prod")
                        nc.vector.scalar_tensor_tensor(
                            out=prod[:],
                            in0=t[:, g * D:(g + 1) * D],
                            scalar=1.0,
                            in1=w_rep,
                            op0=ALU.mult,
                            op1=ALU.mult,
                            accum_out=energN[:, dve_g0 + g:dve_g0 + g + 1],
                        )
                    dve_g0 += ng

            # exp of the DVE energies into the tail columns of outv
            nc.scalar.activation(
                outv[:, B_PE:B], energN[:], AF.Exp, scale=1.0,
            )
            # single contiguous 251KB store
            nc.scalar.dma_start(out_r, outv[:])

    nc.compile()
    return nc


def _get_nc():
    global _nc_cache
    if _nc_cache is None:
        _nc_cache = build_nc()
    return _nc_cache


def make_in_maps(sites, attn_w):
    import ml_dtypes

    bf = ml_dtypes.bfloat16
    sites = np.asarray(sites, dtype=np.float32)
    w = np.asarray(attn_w, dtype=np.float32)[D:2 * D].astype(bf)

    wt = np.zeros((P, 512), dtype=bf)
    wt[:, 0] = w[0:P]
    wt[:, 1] = w[P:2 * P]
    wt[:, 2:2 + D] = w[None, :]          # broadcast copy for the DVE path

    sp = np.zeros((N_PAD, D), dtype=bf)
    sp[:N] = sites.astype(bf)

    maps = []
    for c in range(N_CORES):
        shard = sp[c * SITES_CORE:(c + 1) * SITES_CORE]
        # PE region: [b, m, h, p] -> [p, b, h, m]
        R = shard[:S_PE].reshape(B_PE, P, 2, P)
        X = np.ascontiguousarray(R.transpose(3, 0, 2, 1)).reshape(P, B_PE * 2 * P)
        # DVE region: natural layout, site = S_PE + p*G_DVE + g
        XN = np.ascontiguousarray(
            shard[S_PE:].reshape(P, G_DVE * D)
        )
        maps.append({"sitesT": X, "sitesN": XN, "wt": wt})
    return maps


def unshard(core_outs):
    """core_outs: list of 8 [P*B] f32 arrays of exp(energy) -> full softmax."""
    parts = []
    for c in range(N_CORES):
        A = np.asarray(core_outs[c], dtype=np.float32).reshape(P, B)
        pe = A[:, :B_PE].T.reshape(-1)       # site = b*128 + m
        dv = A[:, B_PE:].reshape(-1)         # site = S_PE + p*G_DVE + g
        parts.append(pe)
        parts.append(dv)
    full = np.concatenate(parts)[:N]
    S = full.sum(dtype=np.float64)
    return (full / S).astype(np.float32)


def kernel(local, sites, attn_w, attn_b):
    from concourse.bass_utils import run_bass_kernel_spmd

    nc = _get_nc()
    in_maps = make_in_maps(sites, attn_w)
    res = run_bass_kernel_spmd(nc, in_maps, list(range(N_CORES)))
    return unshard([res.results[c]["out"] for c in range(N_CORES)])
